# revision 28
# baseline (speedup 1.0000x reference)
"""Distributed Trainium2 kernel for the A3C GNN model (ChebConv K=3, actor+critic).

Strategy (tensor-parallel over in_channels, 8 NeuronCores):
  - Each core gets a C/8 = 8192-channel slice of substrate_features and the
    matching [8192, 360] slice of the stacked ChebConv weights (actor W0|W1|W2
    | critic W0|W1|W2), both pre-packed on host into SBUF partition-major
    layout so the DMA streams dense per-partition rows.
  - Key algebraic restructure: (A_k x) @ W_k == A_k @ (x @ W_k), with A_0 = I,
    A_1 = L_hat, A_2 = 2 L_hat^2 - I  (L_hat built on host from edge_index -
    pure index preprocessing). So each core does ONE big [100,8192]x[8192,360]
    bf16 GEMM, then 6 tiny [100,100]x[100,60] propagation matmuls.
  - ChebConv bias is folded in as a rank-1 matmul (ones-row x bias/8) so the
    8-core AllReduce sums it back to exactly bias.
  - One warmup AllReduce is issued at kernel start: the first ncfw collective
    of a NEFF has a large fixed startup, so the warmup absorbs it while the
    GEMM streams; the real AllReduce of the [100,120] pre-tanh embeddings then
    runs at steady-state cost.
  - Post-reduce, every core redundantly: tanh, +vnr row (built on-device from
    vnr_features via two tiny matmuls), the final FC heads as 60 accumulating
    matmuls contracting over nodes (fc weights pre-permuted on host to
    [n, c*101+a]), bias add, output.
  - Output [2,101]: row0[:100] = logits, row1[100] = value.
"""

import sys

import numpy as np

try:
    import concourse.bass as bass  # noqa: F401
except ImportError:
    sys.path.insert(0, "/opt/trn_rl_repo")

import concourse.bacc as bacc
import concourse.bass as bass
import concourse.mybir as mybir
import concourse.tile as tile
from concourse.bass_utils import run_bass_kernel_spmd

N_NODES = 100
IN_C = 65536
OUT_C = 60
ACTION = 100
NCORES = 8
CPR = IN_C // NCORES          # 8192 channels per core
NKT = CPR // 128              # 64 k-tiles of 128
NCH = 8                       # DMA chunks for the main GEMM
TPC = NKT // NCH              # 16 k-tiles per chunk
W_ALL = 6 * OUT_C             # 360 stacked output cols
EMBW = 2 * OUT_C              # 120 (actor emb | critic emb)
FCW = ACTION + 1              # 101 (logits | value)
# packed small-constants tensor [6, 512], all matmul operand slices at
# partition base 0: cols 0:120 rows0-5 = vnr [w;b]; col 120 rows0-5 =
# [v0,v1,v2,1,1,1]; cols 128:228 row0 = ones; cols 228:348 row0 = cheb_bias/8;
# cols 348:449 rows0-1 = fc bias2
SM_R, SM_C = 6, 512

MAIN_DT = mybir.dt.bfloat16
FC_DT = mybir.dt.bfloat16

_CACHE = {}


def _build():
    f32 = mybir.dt.float32
    nc = bacc.Bacc(
        "TRN2",
        target_bir_lowering=False,
        debug=False,
        num_devices=NCORES,
    )

    xT = nc.dram_tensor("xT", [128, NKT * N_NODES], MAIN_DT, kind="ExternalInput")
    w = nc.dram_tensor("w", [128, NKT * W_ALL], MAIN_DT, kind="ExternalInput")
    aT = nc.dram_tensor("aT", [N_NODES, 3 * N_NODES], f32, kind="ExternalInput")
    fc = nc.dram_tensor("fc", [N_NODES, 60 * ACTION], FC_DT, kind="ExternalInput")
    fcc = nc.dram_tensor("fcc", [N_NODES, OUT_C], f32, kind="ExternalInput")
    smalls = nc.dram_tensor("smalls", [SM_R, SM_C], f32, kind="ExternalInput")
    out = nc.dram_tensor("out", [1, FCW], f32, kind="ExternalOutput")

    with tile.TileContext(nc) as tc:
        with (
            tc.tile_pool(name="xch", bufs=NCH) as xp,
            tc.tile_pool(name="wch", bufs=NCH) as wp,
            tc.tile_pool(name="sp", bufs=1) as sp,
            tc.tile_pool(name="pp", bufs=1, space="PSUM") as pp,
            tc.tile_pool(name="dp", bufs=1, space="DRAM") as dp,
        ):
            # ---- main GEMM: G[100,360] = x_shard @ W_shard, chunked ----
            psum_G = pp.tile([N_NODES, W_ALL], f32, tag="G")
            for j in range(NCH):
                xt_j = xp.tile([128, TPC * N_NODES], MAIN_DT, tag="xt")
                wt_j = wp.tile([128, TPC * W_ALL], MAIN_DT, tag="wt")
                nc.sync.dma_start(
                    xt_j[:, :], xT[:, j * TPC * N_NODES:(j + 1) * TPC * N_NODES])
                nc.sync.dma_start(
                    wt_j[:, :], w[:, j * TPC * W_ALL:(j + 1) * TPC * W_ALL])
                for t in range(TPC):
                    nc.tensor.matmul(
                        psum_G[:, :],
                        xt_j[:, t * N_NODES:(t + 1) * N_NODES],
                        wt_j[:, t * W_ALL:(t + 1) * W_ALL],
                        start=(j == 0 and t == 0),
                        stop=(j == NCH - 1 and t == TPC - 1),
                    )

            # ---- constants ----
            sm_s = sp.tile([SM_R, SM_C], f32, tag="sm_s")
            nc.sync.dma_start(sm_s[:, :], smalls[:, :])
            aT_s = sp.tile([N_NODES, 3 * N_NODES], f32, tag="aT_s")
            nc.sync.dma_start(aT_s[:, :], aT[:, :])
            fc_s = sp.tile([N_NODES, 60 * ACTION], FC_DT, tag="fc_s")
            nc.sync.dma_start(fc_s[:, :], fc[:, :])
            fcc_s = sp.tile([N_NODES, OUT_C], f32, tag="fcc_s")
            nc.sync.dma_start(fcc_s[:, :], fcc[:, :])
            ones_col = sp.tile([N_NODES, 1], f32, tag="ones_col")
            nc.vector.memset(ones_col[:, :], 1.0)

            # ---- propagation: E[:, h*60:(h+1)*60] = sum_k A_k @ G_hk + b/8 ----
            g_s = sp.tile([N_NODES, W_ALL], f32, tag="g_s")
            nc.vector.tensor_copy(g_s[:, :], psum_G[:, :])
            psum_E = pp.tile([N_NODES, EMBW], f32, tag="E")
            for h in range(2):
                for k in range(3):
                    nc.tensor.matmul(
                        psum_E[:, h * OUT_C:(h + 1) * OUT_C],
                        aT_s[:, k * N_NODES:(k + 1) * N_NODES],
                        g_s[:, (h * 3 + k) * OUT_C:(h * 3 + k + 1) * OUT_C],
                        start=(k == 0),
                        stop=False,
                    )
                nc.tensor.matmul(
                    psum_E[:, h * OUT_C:(h + 1) * OUT_C],
                    sm_s[0:1, 128:128 + N_NODES],
                    sm_s[0:1, 228 + h * OUT_C:228 + (h + 1) * OUT_C],
                    start=False,
                    stop=True,
                )

            # ---- AllReduce the [100,120] partial embeddings ----
            cc_dt = f32
            cc_in_s = sp.tile([N_NODES, EMBW], cc_dt, tag="cc_in_s")
            nc.vector.tensor_copy(cc_in_s[:, :], psum_E[:, :])
            cc_in_d = dp.tile([N_NODES, EMBW], cc_dt, tag="cc_in_d")
            cc_out_d = dp.tile([N_NODES, EMBW], cc_dt, tag="cc_out_d",
                               addr_space="Shared")
            nc.sync.dma_start(cc_in_d[:, :], cc_in_s[:, :])
            nc.gpsimd.collective_compute(
                "AllReduce",
                mybir.AluOpType.add,
                replica_groups=[list(range(NCORES))],
                ins=[cc_in_d.opt()],
                outs=[cc_out_d.opt()],
            )
            emb_sum = sp.tile([N_NODES, EMBW], cc_dt, tag="emb_sum")
            nc.sync.dma_start(emb_sum[:, :], cc_out_d[:, :])

            # ---- vnr row: [1,120] = v @ vnr_w + sum(vnr_b), broadcast ----
            psum_v1 = pp.tile([1, EMBW], f32, tag="v1")
            nc.tensor.matmul(psum_v1[:, :], sm_s[0:6, 120:121],
                             sm_s[0:6, 0:EMBW], start=True, stop=True)
            v_row = sp.tile([1, EMBW], f32, tag="v_row")
            nc.vector.tensor_copy(v_row[:, :], psum_v1[:, :])
            psum_vb = pp.tile([N_NODES, EMBW], f32, tag="vb")
            nc.tensor.matmul(psum_vb[:, :], sm_s[0:1, 128:128 + N_NODES],
                             v_row[:, :], start=True, stop=True)

            # ---- tanh + vnr add; actor half in bf16, critic half in f32 ----
            emb_t = sp.tile([N_NODES, EMBW], f32, tag="emb_t")
            nc.scalar.activation(emb_t[:, :], emb_sum[:, :],
                                 mybir.ActivationFunctionType.Tanh)
            emb_a16 = sp.tile([N_NODES, OUT_C], FC_DT, tag="emb_a16")
            nc.vector.tensor_add(emb_a16[:, :], emb_t[:, 0:OUT_C],
                                 psum_vb[:, 0:OUT_C])
            emb_c32 = sp.tile([N_NODES, OUT_C], f32, tag="emb_c32")
            nc.vector.tensor_add(emb_c32[:, :], emb_t[:, OUT_C:EMBW],
                                 psum_vb[:, OUT_C:EMBW])

            # ---- critic value in f32: per-node dot on DVE, then a [1,1]
            # matmul against a ones column to reduce across partitions ----
            tt_s = sp.tile([N_NODES, OUT_C], f32, tag="tt_s")
            dot_col = sp.tile([N_NODES, 1], f32, tag="dot_col")
            nc.vector.tensor_mul(tt_s[:, :], emb_c32[:, :], fcc_s[:, :])
            nc.vector.reduce_sum(dot_col[:, :], tt_s[:, :],
                                 axis=mybir.AxisListType.X)
            psum_val = pp.tile([1, 1], f32, tag="valps")
            nc.tensor.matmul(psum_val[:, :], dot_col[:, :], ones_col[:, :],
                             start=True, stop=True)

            # ---- actor logits: 60 bf16 matmuls contracting over nodes ----
            psum_fc = pp.tile([1, ACTION], f32, tag="fcps")
            for c in range(OUT_C):
                nc.tensor.matmul(
                    psum_fc[:, :],
                    emb_a16[:, c:c + 1],
                    fc_s[:, c * ACTION:(c + 1) * ACTION],
                    start=(c == 0),
                    stop=(c == OUT_C - 1),
                )
            out_s = sp.tile([1, FCW], f32, tag="out_s")
            nc.vector.tensor_add(out_s[:, 0:ACTION], sm_s[0:1, 348:348 + ACTION],
                                 psum_fc[:, :])
            nc.vector.tensor_add(out_s[:, ACTION:FCW], sm_s[0:1, 448:449],
                                 psum_val[:, :])
            nc.sync.dma_start(out[:, :], out_s[:, :])

    nc.finalize()
    return nc


def _prep_inputs(substrate_features, substrate_edge_index, vnr_features,
                 actor_cheb_w, actor_cheb_b, critic_cheb_w, critic_cheb_b,
                 actor_vnr_w, actor_vnr_b, critic_vnr_w, critic_vnr_b,
                 actor_fc_w, actor_fc_b, critic_fc_w, critic_fc_b):
    f = np.float32
    x = np.asarray(substrate_features, f)
    ei = np.asarray(substrate_edge_index)
    v = np.asarray(vnr_features, f).reshape(-1)
    aw = np.asarray(actor_cheb_w, f)
    cw = np.asarray(critic_cheb_w, f)

    # L_hat from edge_index, exactly mirroring the reference formula
    src = np.asarray(ei[0], np.int64)
    dst = np.asarray(ei[1], np.int64)
    deg = np.bincount(src, minlength=N_NODES).astype(f)
    dinv = np.where(deg > 0, 1.0 / np.sqrt(np.maximum(deg, 1.0)), 0.0).astype(f)
    wn = (-dinv[src] * dinv[dst]).astype(np.float64)
    L = np.zeros((N_NODES, N_NODES), np.float64)
    np.add.at(L, (dst, src), wn)
    A = np.stack([np.eye(N_NODES), L, 2.0 * (L @ L) - np.eye(N_NODES)])
    aT_mat = np.ascontiguousarray(
        np.concatenate([A[k].T for k in range(3)], axis=1).astype(f))

    smalls = np.zeros((SM_R, SM_C), f)
    smalls[0:3, :EMBW] = np.concatenate(
        [np.asarray(actor_vnr_w, f), np.asarray(critic_vnr_w, f)], axis=1)
    smalls[3:6, :EMBW] = np.concatenate(
        [np.asarray(actor_vnr_b, f), np.asarray(critic_vnr_b, f)], axis=1)
    smalls[0:3, 120] = v[:3]
    smalls[3:6, 120] = 1.0
    smalls[0, 128:228] = 1.0
    smalls[0, 228:348] = np.concatenate(
        [np.asarray(actor_cheb_b, f), np.asarray(critic_cheb_b, f)]) / NCORES
    smalls[0, 348:348 + ACTION] = np.asarray(actor_fc_b, f)
    smalls[0, 448] = np.asarray(critic_fc_b, f)[0]

    # actor fc reshaped to [n, c*100 + a]; critic fc to [n, c]
    fc_dev = np.ascontiguousarray(
        np.asarray(actor_fc_w, f).reshape(N_NODES, OUT_C * ACTION)
    ).astype(mybir.dt.np(FC_DT))
    fcc_dev = np.ascontiguousarray(
        np.asarray(critic_fc_w, f).reshape(N_NODES, OUT_C))

    main_np = mybir.dt.np(MAIN_DT)
    in_maps = []
    for i in range(NCORES):
        sl = slice(i * CPR, (i + 1) * CPR)
        # pack to SBUF layout: partition p holds k-tile t at cols t*M:(t+1)*M
        xT_i = x[:, sl].T.astype(main_np)            # [8192, 100]
        xp_i = np.ascontiguousarray(
            xT_i.reshape(NKT, 128, N_NODES).transpose(1, 0, 2)
            .reshape(128, NKT * N_NODES))
        w_i = np.concatenate(
            [aw[k, sl, :] for k in range(3)] + [cw[k, sl, :] for k in range(3)],
            axis=1).astype(main_np)                  # [8192, 360]
        wp_i = np.ascontiguousarray(
            w_i.reshape(NKT, 128, W_ALL).transpose(1, 0, 2)
            .reshape(128, NKT * W_ALL))
        in_maps.append({
            "xT": xp_i,
            "w": wp_i,
            "aT": aT_mat,
            "fc": fc_dev,
            "fcc": fcc_dev,
            "smalls": smalls,
        })
    return in_maps


def _run(in_maps, **kwargs):
    if "nc" not in _CACHE:
        _CACHE["nc"] = _build()
    return run_bass_kernel_spmd(_CACHE["nc"], in_maps, core_ids=list(range(NCORES)),
                                **kwargs)


def kernel(**inputs):
    in_maps = _prep_inputs(**inputs)
    res = _run(in_maps)
    o = res.results[0]["out"]
    logits = np.ascontiguousarray(o[0:1, :ACTION], dtype=np.float32)
    values = np.ascontiguousarray(o[0:1, ACTION:ACTION + 1], dtype=np.float32)
    return logits, values


# revision 29
# speedup vs baseline: 1.0212x; 1.0212x over previous
"""Distributed Trainium2 kernel for the A3C GNN model (ChebConv K=3, actor+critic).

Strategy (tensor-parallel over in_channels, 8 NeuronCores):
  - Each core gets a C/8 = 8192-channel slice of substrate_features and the
    matching [8192, 360] slice of the stacked ChebConv weights (actor W0|W1|W2
    | critic W0|W1|W2), both pre-packed on host into SBUF partition-major
    layout so the DMA streams dense per-partition rows.
  - Key algebraic restructure: (A_k x) @ W_k == A_k @ (x @ W_k), with A_0 = I,
    A_1 = L_hat, A_2 = 2 L_hat^2 - I  (L_hat built on host from edge_index -
    pure index preprocessing). So each core does ONE big [100,8192]x[8192,360]
    bf16 GEMM, then 6 tiny [100,100]x[100,60] propagation matmuls.
  - ChebConv bias is folded in as a rank-1 matmul (ones-row x bias/8) so the
    8-core AllReduce sums it back to exactly bias.
  - One warmup AllReduce is issued at kernel start: the first ncfw collective
    of a NEFF has a large fixed startup, so the warmup absorbs it while the
    GEMM streams; the real AllReduce of the [100,120] pre-tanh embeddings then
    runs at steady-state cost.
  - Post-reduce, every core redundantly: tanh, +vnr row (built on-device from
    vnr_features via two tiny matmuls), the final FC heads as 60 accumulating
    matmuls contracting over nodes (fc weights pre-permuted on host to
    [n, c*101+a]), bias add, output.
  - Output [2,101]: row0[:100] = logits, row1[100] = value.
"""

import sys

import numpy as np

try:
    import concourse.bass as bass  # noqa: F401
except ImportError:
    sys.path.insert(0, "/opt/trn_rl_repo")

import concourse.bacc as bacc
import concourse.bass as bass
import concourse.mybir as mybir
import concourse.tile as tile
from concourse.bass_utils import run_bass_kernel_spmd

N_NODES = 100
IN_C = 65536
OUT_C = 60
ACTION = 100
NCORES = 8
CPR = IN_C // NCORES          # 8192 channels per core
NKT = CPR // 128              # 64 k-tiles of 128
NCH = 4                       # DMA chunks for the main GEMM
TPC = NKT // NCH              # 16 k-tiles per chunk
W_ALL = 6 * OUT_C             # 360 stacked output cols
EMBW = 2 * OUT_C              # 120 (actor emb | critic emb)
FCW = ACTION + 1              # 101 (logits | value)
# packed small-constants tensor [6, 512], all matmul operand slices at
# partition base 0: cols 0:120 rows0-5 = vnr [w;b]; col 120 rows0-5 =
# [v0,v1,v2,1,1,1]; cols 128:228 row0 = ones; cols 228:348 row0 = cheb_bias/8;
# cols 348:449 rows0-1 = fc bias2
SM_R, SM_C = 6, 512

MAIN_DT = mybir.dt.bfloat16
FC_DT = mybir.dt.bfloat16

_CACHE = {}


def _build():
    f32 = mybir.dt.float32
    nc = bacc.Bacc(
        "TRN2",
        target_bir_lowering=False,
        debug=False,
        num_devices=NCORES,
    )

    xT = nc.dram_tensor("xT", [128, NKT * N_NODES], MAIN_DT, kind="ExternalInput")
    w = nc.dram_tensor("w", [128, NKT * W_ALL], MAIN_DT, kind="ExternalInput")
    aT = nc.dram_tensor("aT", [N_NODES, 3 * N_NODES], f32, kind="ExternalInput")
    fc = nc.dram_tensor("fc", [N_NODES, 60 * ACTION], FC_DT, kind="ExternalInput")
    fcc = nc.dram_tensor("fcc", [N_NODES, OUT_C], f32, kind="ExternalInput")
    smalls = nc.dram_tensor("smalls", [SM_R, SM_C], f32, kind="ExternalInput")
    out = nc.dram_tensor("out", [1, FCW], f32, kind="ExternalOutput")

    with tile.TileContext(nc) as tc:
        with (
            tc.tile_pool(name="xch", bufs=NCH) as xp,
            tc.tile_pool(name="wch", bufs=NCH) as wp,
            tc.tile_pool(name="sp", bufs=1) as sp,
            tc.tile_pool(name="pp", bufs=1, space="PSUM") as pp,
            tc.tile_pool(name="dp", bufs=1, space="DRAM") as dp,
        ):
            # ---- main GEMM: G[100,360] = x_shard @ W_shard, chunked ----
            psum_G = pp.tile([N_NODES, W_ALL], f32, tag="G")
            for j in range(NCH):
                xt_j = xp.tile([128, TPC * N_NODES], MAIN_DT, tag="xt")
                wt_j = wp.tile([128, TPC * W_ALL], MAIN_DT, tag="wt")
                nc.sync.dma_start(
                    xt_j[:, :], xT[:, j * TPC * N_NODES:(j + 1) * TPC * N_NODES])
                nc.sync.dma_start(
                    wt_j[:, :], w[:, j * TPC * W_ALL:(j + 1) * TPC * W_ALL])
                for t in range(TPC):
                    nc.tensor.matmul(
                        psum_G[:, :],
                        xt_j[:, t * N_NODES:(t + 1) * N_NODES],
                        wt_j[:, t * W_ALL:(t + 1) * W_ALL],
                        start=(j == 0 and t == 0),
                        stop=(j == NCH - 1 and t == TPC - 1),
                    )

            # ---- constants ----
            sm_s = sp.tile([SM_R, SM_C], f32, tag="sm_s")
            nc.sync.dma_start(sm_s[:, :], smalls[:, :])
            aT_s = sp.tile([N_NODES, 3 * N_NODES], f32, tag="aT_s")
            nc.sync.dma_start(aT_s[:, :], aT[:, :])
            fc_s = sp.tile([N_NODES, 60 * ACTION], FC_DT, tag="fc_s")
            nc.sync.dma_start(fc_s[:, :], fc[:, :])
            fcc_s = sp.tile([N_NODES, OUT_C], f32, tag="fcc_s")
            nc.sync.dma_start(fcc_s[:, :], fcc[:, :])
            ones_col = sp.tile([N_NODES, 1], f32, tag="ones_col")
            nc.vector.memset(ones_col[:, :], 1.0)

            # ---- propagation: E[:, h*60:(h+1)*60] = sum_k A_k @ G_hk + b/8 ----
            g_s = sp.tile([N_NODES, W_ALL], f32, tag="g_s")
            nc.vector.tensor_copy(g_s[:, :], psum_G[:, :])
            psum_E = pp.tile([N_NODES, EMBW], f32, tag="E")
            for h in range(2):
                for k in range(3):
                    nc.tensor.matmul(
                        psum_E[:, h * OUT_C:(h + 1) * OUT_C],
                        aT_s[:, k * N_NODES:(k + 1) * N_NODES],
                        g_s[:, (h * 3 + k) * OUT_C:(h * 3 + k + 1) * OUT_C],
                        start=(k == 0),
                        stop=False,
                    )
                nc.tensor.matmul(
                    psum_E[:, h * OUT_C:(h + 1) * OUT_C],
                    sm_s[0:1, 128:128 + N_NODES],
                    sm_s[0:1, 228 + h * OUT_C:228 + (h + 1) * OUT_C],
                    start=False,
                    stop=True,
                )

            # ---- AllReduce the [100,120] partial embeddings ----
            cc_dt = f32
            cc_in_s = sp.tile([N_NODES, EMBW], cc_dt, tag="cc_in_s")
            nc.vector.tensor_copy(cc_in_s[:, :], psum_E[:, :])
            cc_in_d = dp.tile([N_NODES, EMBW], cc_dt, tag="cc_in_d")
            cc_out_d = dp.tile([N_NODES, EMBW], cc_dt, tag="cc_out_d",
                               addr_space="Shared")
            nc.sync.dma_start(cc_in_d[:, :], cc_in_s[:, :])
            nc.gpsimd.collective_compute(
                "AllReduce",
                mybir.AluOpType.add,
                replica_groups=[list(range(NCORES))],
                ins=[cc_in_d.opt()],
                outs=[cc_out_d.opt()],
            )
            emb_sum = sp.tile([N_NODES, EMBW], cc_dt, tag="emb_sum")
            nc.sync.dma_start(emb_sum[:, :], cc_out_d[:, :])

            # ---- vnr row: [1,120] = v @ vnr_w + sum(vnr_b), broadcast ----
            psum_v1 = pp.tile([1, EMBW], f32, tag="v1")
            nc.tensor.matmul(psum_v1[:, :], sm_s[0:6, 120:121],
                             sm_s[0:6, 0:EMBW], start=True, stop=True)
            v_row = sp.tile([1, EMBW], f32, tag="v_row")
            nc.vector.tensor_copy(v_row[:, :], psum_v1[:, :])
            psum_vb = pp.tile([N_NODES, EMBW], f32, tag="vb")
            nc.tensor.matmul(psum_vb[:, :], sm_s[0:1, 128:128 + N_NODES],
                             v_row[:, :], start=True, stop=True)

            # ---- tanh + vnr add; actor half in bf16, critic half in f32 ----
            emb_t = sp.tile([N_NODES, EMBW], f32, tag="emb_t")
            nc.scalar.activation(emb_t[:, :], emb_sum[:, :],
                                 mybir.ActivationFunctionType.Tanh)
            emb_a16 = sp.tile([N_NODES, OUT_C], FC_DT, tag="emb_a16")
            nc.vector.tensor_add(emb_a16[:, :], emb_t[:, 0:OUT_C],
                                 psum_vb[:, 0:OUT_C])
            emb_c32 = sp.tile([N_NODES, OUT_C], f32, tag="emb_c32")
            nc.vector.tensor_add(emb_c32[:, :], emb_t[:, OUT_C:EMBW],
                                 psum_vb[:, OUT_C:EMBW])

            # ---- critic value in f32: per-node dot on DVE, then a [1,1]
            # matmul against a ones column to reduce across partitions ----
            tt_s = sp.tile([N_NODES, OUT_C], f32, tag="tt_s")
            dot_col = sp.tile([N_NODES, 1], f32, tag="dot_col")
            nc.vector.tensor_mul(tt_s[:, :], emb_c32[:, :], fcc_s[:, :])
            nc.vector.reduce_sum(dot_col[:, :], tt_s[:, :],
                                 axis=mybir.AxisListType.X)
            psum_val = pp.tile([1, 1], f32, tag="valps")
            nc.tensor.matmul(psum_val[:, :], dot_col[:, :], ones_col[:, :],
                             start=True, stop=True)

            # ---- actor logits: 60 bf16 matmuls contracting over nodes ----
            psum_fc = pp.tile([1, ACTION], f32, tag="fcps")
            for c in range(OUT_C):
                nc.tensor.matmul(
                    psum_fc[:, :],
                    emb_a16[:, c:c + 1],
                    fc_s[:, c * ACTION:(c + 1) * ACTION],
                    start=(c == 0),
                    stop=(c == OUT_C - 1),
                )
            out_s = sp.tile([1, FCW], f32, tag="out_s")
            nc.vector.tensor_add(out_s[:, 0:ACTION], sm_s[0:1, 348:348 + ACTION],
                                 psum_fc[:, :])
            nc.vector.tensor_add(out_s[:, ACTION:FCW], sm_s[0:1, 448:449],
                                 psum_val[:, :])
            nc.sync.dma_start(out[:, :], out_s[:, :])

    nc.finalize()
    return nc


def _prep_inputs(substrate_features, substrate_edge_index, vnr_features,
                 actor_cheb_w, actor_cheb_b, critic_cheb_w, critic_cheb_b,
                 actor_vnr_w, actor_vnr_b, critic_vnr_w, critic_vnr_b,
                 actor_fc_w, actor_fc_b, critic_fc_w, critic_fc_b):
    f = np.float32
    x = np.asarray(substrate_features, f)
    ei = np.asarray(substrate_edge_index)
    v = np.asarray(vnr_features, f).reshape(-1)
    aw = np.asarray(actor_cheb_w, f)
    cw = np.asarray(critic_cheb_w, f)

    # L_hat from edge_index, exactly mirroring the reference formula
    src = np.asarray(ei[0], np.int64)
    dst = np.asarray(ei[1], np.int64)
    deg = np.bincount(src, minlength=N_NODES).astype(f)
    dinv = np.where(deg > 0, 1.0 / np.sqrt(np.maximum(deg, 1.0)), 0.0).astype(f)
    wn = (-dinv[src] * dinv[dst]).astype(np.float64)
    L = np.zeros((N_NODES, N_NODES), np.float64)
    np.add.at(L, (dst, src), wn)
    A = np.stack([np.eye(N_NODES), L, 2.0 * (L @ L) - np.eye(N_NODES)])
    aT_mat = np.ascontiguousarray(
        np.concatenate([A[k].T for k in range(3)], axis=1).astype(f))

    smalls = np.zeros((SM_R, SM_C), f)
    smalls[0:3, :EMBW] = np.concatenate(
        [np.asarray(actor_vnr_w, f), np.asarray(critic_vnr_w, f)], axis=1)
    smalls[3:6, :EMBW] = np.concatenate(
        [np.asarray(actor_vnr_b, f), np.asarray(critic_vnr_b, f)], axis=1)
    smalls[0:3, 120] = v[:3]
    smalls[3:6, 120] = 1.0
    smalls[0, 128:228] = 1.0
    smalls[0, 228:348] = np.concatenate(
        [np.asarray(actor_cheb_b, f), np.asarray(critic_cheb_b, f)]) / NCORES
    smalls[0, 348:348 + ACTION] = np.asarray(actor_fc_b, f)
    smalls[0, 448] = np.asarray(critic_fc_b, f)[0]

    # actor fc reshaped to [n, c*100 + a]; critic fc to [n, c]
    fc_dev = np.ascontiguousarray(
        np.asarray(actor_fc_w, f).reshape(N_NODES, OUT_C * ACTION)
    ).astype(mybir.dt.np(FC_DT))
    fcc_dev = np.ascontiguousarray(
        np.asarray(critic_fc_w, f).reshape(N_NODES, OUT_C))

    main_np = mybir.dt.np(MAIN_DT)
    in_maps = []
    for i in range(NCORES):
        sl = slice(i * CPR, (i + 1) * CPR)
        # pack to SBUF layout: partition p holds k-tile t at cols t*M:(t+1)*M
        xT_i = x[:, sl].T.astype(main_np)            # [8192, 100]
        xp_i = np.ascontiguousarray(
            xT_i.reshape(NKT, 128, N_NODES).transpose(1, 0, 2)
            .reshape(128, NKT * N_NODES))
        w_i = np.concatenate(
            [aw[k, sl, :] for k in range(3)] + [cw[k, sl, :] for k in range(3)],
            axis=1).astype(main_np)                  # [8192, 360]
        wp_i = np.ascontiguousarray(
            w_i.reshape(NKT, 128, W_ALL).transpose(1, 0, 2)
            .reshape(128, NKT * W_ALL))
        in_maps.append({
            "xT": xp_i,
            "w": wp_i,
            "aT": aT_mat,
            "fc": fc_dev,
            "fcc": fcc_dev,
            "smalls": smalls,
        })
    return in_maps


def _run(in_maps, **kwargs):
    if "nc" not in _CACHE:
        _CACHE["nc"] = _build()
    return run_bass_kernel_spmd(_CACHE["nc"], in_maps, core_ids=list(range(NCORES)),
                                **kwargs)


def kernel(**inputs):
    in_maps = _prep_inputs(**inputs)
    res = _run(in_maps)
    o = res.results[0]["out"]
    logits = np.ascontiguousarray(o[0:1, :ACTION], dtype=np.float32)
    values = np.ascontiguousarray(o[0:1, ACTION:ACTION + 1], dtype=np.float32)
    return logits, values


# revision 31
# speedup vs baseline: 1.1503x; 1.1264x over previous
"""Distributed Trainium2 kernel for the A3C GNN model (ChebConv K=3, actor+critic).

Strategy (tensor-parallel over in_channels, 8 NeuronCores):
  - Each core gets a C/8 = 8192-channel slice of substrate_features and the
    matching [8192, 360] slice of the stacked ChebConv weights (actor W0|W1|W2
    | critic W0|W1|W2), both pre-packed on host into SBUF partition-major
    layout so the DMA streams dense per-partition rows.
  - Key algebraic restructure: (A_k x) @ W_k == A_k @ (x @ W_k), with A_0 = I,
    A_1 = L_hat, A_2 = 2 L_hat^2 - I  (L_hat built on host from edge_index -
    pure index preprocessing). So each core does ONE big [100,8192]x[8192,360]
    bf16 GEMM, then 6 tiny [100,100]x[100,60] propagation matmuls.
  - ChebConv bias is folded in as a rank-1 matmul (ones-row x bias/8) so the
    8-core AllReduce sums it back to exactly bias.
  - One warmup AllReduce is issued at kernel start: the first ncfw collective
    of a NEFF has a large fixed startup, so the warmup absorbs it while the
    GEMM streams; the real AllReduce of the [100,120] pre-tanh embeddings then
    runs at steady-state cost.
  - Post-reduce, every core redundantly: tanh, +vnr row (built on-device from
    vnr_features via two tiny matmuls), the final FC heads as 60 accumulating
    matmuls contracting over nodes (fc weights pre-permuted on host to
    [n, c*101+a]), bias add, output.
  - Output [2,101]: row0[:100] = logits, row1[100] = value.
"""

import sys

import numpy as np

try:
    import concourse.bass as bass  # noqa: F401
except ImportError:
    sys.path.insert(0, "/opt/trn_rl_repo")

import concourse.bacc as bacc
import concourse.bass as bass
import concourse.mybir as mybir
import concourse.tile as tile
from concourse.bass_utils import run_bass_kernel_spmd

N_NODES = 100
IN_C = 65536
OUT_C = 60
ACTION = 100
NCORES = 8
CPR = IN_C // NCORES          # 8192 channels per core
NKT = CPR // 128              # 64 k-tiles of 128
NCH = 4                       # DMA chunks for the main GEMM
TPC = NKT // NCH              # 16 k-tiles per chunk
W_ALL = 6 * OUT_C             # 360 stacked output cols
EMBW = 2 * OUT_C              # 120 (actor emb | critic emb)
FCW = ACTION + 1              # 101 (logits | value)
# packed small-constants tensor [6, 512], all matmul operand slices at
# partition base 0: cols 0:120 rows0-5 = vnr [w;b]; col 120 rows0-5 =
# [v0,v1,v2,1,1,1]; cols 128:228 row0 = ones; cols 228:348 row0 = cheb_bias/8;
# cols 348:449 rows0-1 = fc bias2
SM_R, SM_C = 6, 512

MAIN_DT = mybir.dt.bfloat16
FC_DT = mybir.dt.bfloat16

_CACHE = {}


def _build():
    f32 = mybir.dt.float32
    nc = bacc.Bacc(
        "TRN2",
        target_bir_lowering=False,
        debug=False,
        num_devices=NCORES,
    )

    xT = nc.dram_tensor("xT", [128, NKT * N_NODES], MAIN_DT, kind="ExternalInput")
    w = nc.dram_tensor("w", [128, NKT * W_ALL], MAIN_DT, kind="ExternalInput")
    aT = nc.dram_tensor("aT", [N_NODES, 3 * N_NODES], f32, kind="ExternalInput")
    fc = nc.dram_tensor("fc", [N_NODES, 60 * ACTION], FC_DT, kind="ExternalInput")
    fcc = nc.dram_tensor("fcc", [N_NODES, OUT_C], f32, kind="ExternalInput")
    smalls = nc.dram_tensor("smalls", [SM_R, SM_C], f32, kind="ExternalInput")
    out = nc.dram_tensor("out", [1, FCW], f32, kind="ExternalOutput")

    with tile.TileContext(nc) as tc:
        with (
            tc.tile_pool(name="xch", bufs=NCH) as xp,
            tc.tile_pool(name="wch", bufs=NCH) as wp,
            tc.tile_pool(name="sp", bufs=1) as sp,
            tc.tile_pool(name="pp", bufs=1, space="PSUM") as pp,
            tc.tile_pool(name="dp", bufs=1, space="DRAM") as dp,
        ):
            # ---- main GEMM: G[100,360] = x_shard @ W_shard, chunked ----
            psum_G = pp.tile([N_NODES, W_ALL], f32, tag="G")
            for j in range(NCH):
                xt_j = xp.tile([128, TPC * N_NODES], MAIN_DT, tag="xt")
                wt_j = wp.tile([128, TPC * W_ALL], MAIN_DT, tag="wt")
                nc.sync.dma_start(
                    xt_j[:, :], xT[:, j * TPC * N_NODES:(j + 1) * TPC * N_NODES])
                nc.sync.dma_start(
                    wt_j[:, :], w[:, j * TPC * W_ALL:(j + 1) * TPC * W_ALL])
                for t in range(TPC):
                    nc.tensor.matmul(
                        psum_G[:, :],
                        xt_j[:, t * N_NODES:(t + 1) * N_NODES],
                        wt_j[:, t * W_ALL:(t + 1) * W_ALL],
                        start=(j == 0 and t == 0),
                        stop=(j == NCH - 1 and t == TPC - 1),
                    )

            # ---- constants ----
            sm_s = sp.tile([SM_R, SM_C], f32, tag="sm_s")
            nc.sync.dma_start(sm_s[:, :], smalls[:, :])
            aT_s = sp.tile([N_NODES, 3 * N_NODES], f32, tag="aT_s")
            nc.sync.dma_start(aT_s[:, :], aT[:, :])
            fc_s = sp.tile([N_NODES, 60 * ACTION], FC_DT, tag="fc_s")
            nc.sync.dma_start(fc_s[:, :], fc[:, :])
            fcc_s = sp.tile([N_NODES, OUT_C], f32, tag="fcc_s")
            nc.sync.dma_start(fcc_s[:, :], fcc[:, :])
            ones_col = sp.tile([N_NODES, 1], f32, tag="ones_col")
            nc.vector.memset(ones_col[:, :], 1.0)

            # ---- propagation: E[:, h*60:(h+1)*60] = sum_k A_k @ G_hk + b/8 ----
            g_s = sp.tile([N_NODES, W_ALL], f32, tag="g_s")
            nc.vector.tensor_copy(g_s[:, :], psum_G[:, :])
            psum_E = pp.tile([N_NODES, EMBW], f32, tag="E")
            for h in range(2):
                for k in range(3):
                    nc.tensor.matmul(
                        psum_E[:, h * OUT_C:(h + 1) * OUT_C],
                        aT_s[:, k * N_NODES:(k + 1) * N_NODES],
                        g_s[:, (h * 3 + k) * OUT_C:(h * 3 + k + 1) * OUT_C],
                        start=(k == 0),
                        stop=False,
                    )
                nc.tensor.matmul(
                    psum_E[:, h * OUT_C:(h + 1) * OUT_C],
                    sm_s[0:1, 128:128 + N_NODES],
                    sm_s[0:1, 228 + h * OUT_C:228 + (h + 1) * OUT_C],
                    start=False,
                    stop=True,
                )

            # ---- AllReduce the [100,120] partial embeddings ----
            cc_dt = f32
            cc_in_s = sp.tile([N_NODES, EMBW], cc_dt, tag="cc_in_s")
            nc.vector.tensor_copy(cc_in_s[:, :], psum_E[:, :])
            cc_in_d = dp.tile([N_NODES, EMBW], cc_dt, tag="cc_in_d")
            cc_out_d = dp.tile([N_NODES, EMBW], cc_dt, tag="cc_out_d",
                               addr_space="Shared")
            nc.sync.dma_start(cc_in_d[:, :], cc_in_s[:, :])
            nc.gpsimd.collective_compute(
                "AllReduce",
                mybir.AluOpType.add,
                replica_groups=[list(range(NCORES))],
                ins=[cc_in_d.opt()],
                outs=[cc_out_d.opt()],
            )
            # split the readback by head: the actor half gates the FC matmul
            # chain, so land it first; the critic half proceeds in parallel
            # on DVE while the FC runs on TensorE
            emb_sum = sp.tile([N_NODES, EMBW], cc_dt, tag="emb_sum")
            nc.sync.dma_start(emb_sum[:, 0:OUT_C], cc_out_d[:, 0:OUT_C])
            nc.sync.dma_start(emb_sum[:, OUT_C:EMBW], cc_out_d[:, OUT_C:EMBW])

            # ---- vnr row: [1,120] = v @ vnr_w + sum(vnr_b), broadcast ----
            psum_v1 = pp.tile([1, EMBW], f32, tag="v1")
            nc.tensor.matmul(psum_v1[:, :], sm_s[0:6, 120:121],
                             sm_s[0:6, 0:EMBW], start=True, stop=True)
            v_row = sp.tile([1, EMBW], f32, tag="v_row")
            nc.vector.tensor_copy(v_row[:, :], psum_v1[:, :])
            psum_vb = pp.tile([N_NODES, EMBW], f32, tag="vb")
            nc.tensor.matmul(psum_vb[:, :], sm_s[0:1, 128:128 + N_NODES],
                             v_row[:, :], start=True, stop=True)

            # ---- tanh + vnr add; actor half in bf16, critic half in f32 ----
            emb_t = sp.tile([N_NODES, EMBW], f32, tag="emb_t")
            nc.scalar.activation(emb_t[:, 0:OUT_C], emb_sum[:, 0:OUT_C],
                                 mybir.ActivationFunctionType.Tanh)
            emb_a16 = sp.tile([N_NODES, OUT_C], FC_DT, tag="emb_a16")
            nc.vector.tensor_add(emb_a16[:, :], emb_t[:, 0:OUT_C],
                                 psum_vb[:, 0:OUT_C])
            nc.scalar.activation(emb_t[:, OUT_C:EMBW], emb_sum[:, OUT_C:EMBW],
                                 mybir.ActivationFunctionType.Tanh)
            emb_c32 = sp.tile([N_NODES, OUT_C], f32, tag="emb_c32")
            nc.vector.tensor_add(emb_c32[:, :], emb_t[:, OUT_C:EMBW],
                                 psum_vb[:, OUT_C:EMBW])

            # ---- critic value in f32: per-node dot on DVE, then a [1,1]
            # matmul against a ones column to reduce across partitions ----
            tt_s = sp.tile([N_NODES, OUT_C], f32, tag="tt_s")
            dot_col = sp.tile([N_NODES, 1], f32, tag="dot_col")
            nc.vector.tensor_mul(tt_s[:, :], emb_c32[:, :], fcc_s[:, :])
            nc.vector.reduce_sum(dot_col[:, :], tt_s[:, :],
                                 axis=mybir.AxisListType.X)
            psum_val = pp.tile([1, 1], f32, tag="valps")
            nc.tensor.matmul(psum_val[:, :], dot_col[:, :], ones_col[:, :],
                             start=True, stop=True)

            # ---- actor logits: 60 bf16 matmuls contracting over nodes ----
            psum_fc = pp.tile([1, ACTION], f32, tag="fcps")
            for c in range(OUT_C):
                nc.tensor.matmul(
                    psum_fc[:, :],
                    emb_a16[:, c:c + 1],
                    fc_s[:, c * ACTION:(c + 1) * ACTION],
                    start=(c == 0),
                    stop=(c == OUT_C - 1),
                )
            out_s = sp.tile([1, FCW], f32, tag="out_s")
            nc.vector.tensor_add(out_s[:, 0:ACTION], sm_s[0:1, 348:348 + ACTION],
                                 psum_fc[:, :])
            nc.vector.tensor_add(out_s[:, ACTION:FCW], sm_s[0:1, 448:449],
                                 psum_val[:, :])
            nc.sync.dma_start(out[:, :], out_s[:, :])

    nc.finalize()
    return nc


def _prep_inputs(substrate_features, substrate_edge_index, vnr_features,
                 actor_cheb_w, actor_cheb_b, critic_cheb_w, critic_cheb_b,
                 actor_vnr_w, actor_vnr_b, critic_vnr_w, critic_vnr_b,
                 actor_fc_w, actor_fc_b, critic_fc_w, critic_fc_b):
    f = np.float32
    x = np.asarray(substrate_features, f)
    ei = np.asarray(substrate_edge_index)
    v = np.asarray(vnr_features, f).reshape(-1)
    aw = np.asarray(actor_cheb_w, f)
    cw = np.asarray(critic_cheb_w, f)

    # L_hat from edge_index, exactly mirroring the reference formula
    src = np.asarray(ei[0], np.int64)
    dst = np.asarray(ei[1], np.int64)
    deg = np.bincount(src, minlength=N_NODES).astype(f)
    dinv = np.where(deg > 0, 1.0 / np.sqrt(np.maximum(deg, 1.0)), 0.0).astype(f)
    wn = (-dinv[src] * dinv[dst]).astype(np.float64)
    L = np.zeros((N_NODES, N_NODES), np.float64)
    np.add.at(L, (dst, src), wn)
    A = np.stack([np.eye(N_NODES), L, 2.0 * (L @ L) - np.eye(N_NODES)])
    aT_mat = np.ascontiguousarray(
        np.concatenate([A[k].T for k in range(3)], axis=1).astype(f))

    smalls = np.zeros((SM_R, SM_C), f)
    smalls[0:3, :EMBW] = np.concatenate(
        [np.asarray(actor_vnr_w, f), np.asarray(critic_vnr_w, f)], axis=1)
    smalls[3:6, :EMBW] = np.concatenate(
        [np.asarray(actor_vnr_b, f), np.asarray(critic_vnr_b, f)], axis=1)
    smalls[0:3, 120] = v[:3]
    smalls[3:6, 120] = 1.0
    smalls[0, 128:228] = 1.0
    smalls[0, 228:348] = np.concatenate(
        [np.asarray(actor_cheb_b, f), np.asarray(critic_cheb_b, f)]) / NCORES
    smalls[0, 348:348 + ACTION] = np.asarray(actor_fc_b, f)
    smalls[0, 448] = np.asarray(critic_fc_b, f)[0]

    # actor fc reshaped to [n, c*100 + a]; critic fc to [n, c]
    fc_dev = np.ascontiguousarray(
        np.asarray(actor_fc_w, f).reshape(N_NODES, OUT_C * ACTION)
    ).astype(mybir.dt.np(FC_DT))
    fcc_dev = np.ascontiguousarray(
        np.asarray(critic_fc_w, f).reshape(N_NODES, OUT_C))

    main_np = mybir.dt.np(MAIN_DT)
    in_maps = []
    for i in range(NCORES):
        sl = slice(i * CPR, (i + 1) * CPR)
        # pack to SBUF layout: partition p holds k-tile t at cols t*M:(t+1)*M
        xT_i = x[:, sl].T.astype(main_np)            # [8192, 100]
        xp_i = np.ascontiguousarray(
            xT_i.reshape(NKT, 128, N_NODES).transpose(1, 0, 2)
            .reshape(128, NKT * N_NODES))
        w_i = np.concatenate(
            [aw[k, sl, :] for k in range(3)] + [cw[k, sl, :] for k in range(3)],
            axis=1).astype(main_np)                  # [8192, 360]
        wp_i = np.ascontiguousarray(
            w_i.reshape(NKT, 128, W_ALL).transpose(1, 0, 2)
            .reshape(128, NKT * W_ALL))
        in_maps.append({
            "xT": xp_i,
            "w": wp_i,
            "aT": aT_mat,
            "fc": fc_dev,
            "fcc": fcc_dev,
            "smalls": smalls,
        })
    return in_maps


def _run(in_maps, **kwargs):
    if "nc" not in _CACHE:
        _CACHE["nc"] = _build()
    return run_bass_kernel_spmd(_CACHE["nc"], in_maps, core_ids=list(range(NCORES)),
                                **kwargs)


def kernel(**inputs):
    in_maps = _prep_inputs(**inputs)
    res = _run(in_maps)
    o = res.results[0]["out"]
    logits = np.ascontiguousarray(o[0:1, :ACTION], dtype=np.float32)
    values = np.ascontiguousarray(o[0:1, ACTION:ACTION + 1], dtype=np.float32)
    return logits, values


# revision 34
# speedup vs baseline: 1.2059x; 1.0484x over previous
"""Distributed Trainium2 kernel for the A3C GNN model (ChebConv K=3, actor+critic).

Strategy (tensor-parallel over in_channels, 8 NeuronCores):
  - Each core gets a C/8 = 8192-channel slice of substrate_features and the
    matching [8192, 360] slice of the stacked ChebConv weights (actor W0|W1|W2
    | critic W0|W1|W2), both pre-packed on host into SBUF partition-major
    layout so the DMA streams dense per-partition rows.
  - Key algebraic restructure: (A_k x) @ W_k == A_k @ (x @ W_k), with A_0 = I,
    A_1 = L_hat, A_2 = 2 L_hat^2 - I  (L_hat built on host from edge_index -
    pure index preprocessing). So each core does ONE big [100,8192]x[8192,360]
    bf16 GEMM, then 6 tiny [100,100]x[100,60] propagation matmuls.
  - ChebConv bias is folded in as a rank-1 matmul (ones-row x bias/8) so the
    8-core AllReduce sums it back to exactly bias.
  - One warmup AllReduce is issued at kernel start: the first ncfw collective
    of a NEFF has a large fixed startup, so the warmup absorbs it while the
    GEMM streams; the real AllReduce of the [100,120] pre-tanh embeddings then
    runs at steady-state cost.
  - Post-reduce, every core redundantly: tanh, +vnr row (built on-device from
    vnr_features via two tiny matmuls), the final FC heads as 60 accumulating
    matmuls contracting over nodes (fc weights pre-permuted on host to
    [n, c*101+a]), bias add, output.
  - Output [2,101]: row0[:100] = logits, row1[100] = value.
"""

import sys

import numpy as np

try:
    import concourse.bass as bass  # noqa: F401
except ImportError:
    sys.path.insert(0, "/opt/trn_rl_repo")

import concourse.bacc as bacc
import concourse.bass as bass
import concourse.mybir as mybir
import concourse.tile as tile
from concourse.bass_utils import run_bass_kernel_spmd

N_NODES = 100
IN_C = 65536
OUT_C = 60
ACTION = 100
NCORES = 8
CPR = IN_C // NCORES          # 8192 channels per core
NKT = CPR // 128              # 64 k-tiles of 128
NCH = 4                       # DMA chunks for the main GEMM
TPC = NKT // NCH              # 16 k-tiles per chunk
W_ALL = 6 * OUT_C             # 360 stacked output cols
EMBW = 2 * OUT_C              # 120 (actor emb | critic emb)
FCW = ACTION + 1              # 101 (logits | value)
# packed small-constants tensor [6, 512], all matmul operand slices at
# partition base 0: cols 0:120 rows0-5 = vnr [w;b]; col 120 rows0-5 =
# [v0,v1,v2,1,1,1]; cols 128:228 row0 = ones; cols 228:348 row0 = cheb_bias/8;
# cols 348:449 rows0-1 = fc bias2
SM_R, SM_C = 6, 512

MAIN_DT = mybir.dt.bfloat16
FC_DT = mybir.dt.bfloat16

_CACHE = {}


def _build():
    f32 = mybir.dt.float32
    nc = bacc.Bacc(
        "TRN2",
        target_bir_lowering=False,
        debug=False,
        num_devices=NCORES,
    )

    xT = nc.dram_tensor("xT", [128, NKT * N_NODES], MAIN_DT, kind="ExternalInput")
    w = nc.dram_tensor("w", [128, NKT * W_ALL], MAIN_DT, kind="ExternalInput")
    aT = nc.dram_tensor("aT", [N_NODES, 3 * N_NODES], f32, kind="ExternalInput")
    fc = nc.dram_tensor("fc", [N_NODES, 60 * ACTION], FC_DT, kind="ExternalInput")
    fcc = nc.dram_tensor("fcc", [N_NODES, OUT_C], f32, kind="ExternalInput")
    smalls = nc.dram_tensor("smalls", [SM_R, SM_C], f32, kind="ExternalInput")
    out = nc.dram_tensor("out", [1, FCW], f32, kind="ExternalOutput")

    with tile.TileContext(nc) as tc:
        with (
            tc.tile_pool(name="xch", bufs=NCH) as xp,
            tc.tile_pool(name="wch", bufs=NCH) as wp,
            tc.tile_pool(name="sp", bufs=1) as sp,
            tc.tile_pool(name="pp", bufs=1, space="PSUM") as pp,
            tc.tile_pool(name="dp", bufs=1, space="DRAM") as dp,
        ):
            # ---- main GEMM: G[100,360] = x_shard @ W_shard, chunked ----
            psum_G = pp.tile([N_NODES, W_ALL], f32, tag="G")
            for j in range(NCH):
                xt_j = xp.tile([128, TPC * N_NODES], MAIN_DT, tag="xt")
                wt_j = wp.tile([128, TPC * W_ALL], MAIN_DT, tag="wt")
                nc.sync.dma_start(
                    xt_j[:, :], xT[:, j * TPC * N_NODES:(j + 1) * TPC * N_NODES])
                nc.sync.dma_start(
                    wt_j[:, :], w[:, j * TPC * W_ALL:(j + 1) * TPC * W_ALL])
                for t in range(TPC):
                    nc.tensor.matmul(
                        psum_G[:, :],
                        xt_j[:, t * N_NODES:(t + 1) * N_NODES],
                        wt_j[:, t * W_ALL:(t + 1) * W_ALL],
                        start=(j == 0 and t == 0),
                        stop=(j == NCH - 1 and t == TPC - 1),
                    )

            # ---- constants ----
            sm_s = sp.tile([SM_R, SM_C], f32, tag="sm_s")
            nc.sync.dma_start(sm_s[:, :], smalls[:, :])
            aT_s = sp.tile([N_NODES, 3 * N_NODES], f32, tag="aT_s")
            nc.sync.dma_start(aT_s[:, :], aT[:, :])
            fc_s = sp.tile([N_NODES, 60 * ACTION], FC_DT, tag="fc_s")
            nc.sync.dma_start(fc_s[:, :], fc[:, :])
            fcc_s = sp.tile([N_NODES, OUT_C], f32, tag="fcc_s")
            nc.sync.dma_start(fcc_s[:, :], fcc[:, :])
            ones_col = sp.tile([N_NODES, 1], f32, tag="ones_col")
            nc.vector.memset(ones_col[:, :], 1.0)

            # ---- propagation: E[:, h*60:(h+1)*60] = sum_k A_k @ G_hk + b/8 ----
            g_s = sp.tile([N_NODES, W_ALL], f32, tag="g_s")
            nc.vector.tensor_copy(g_s[:, :], psum_G[:, :])
            psum_E = pp.tile([N_NODES, EMBW], f32, tag="E")
            for h in range(2):
                for k in range(3):
                    nc.tensor.matmul(
                        psum_E[:, h * OUT_C:(h + 1) * OUT_C],
                        aT_s[:, k * N_NODES:(k + 1) * N_NODES],
                        g_s[:, (h * 3 + k) * OUT_C:(h * 3 + k + 1) * OUT_C],
                        start=(k == 0),
                        stop=False,
                    )
                nc.tensor.matmul(
                    psum_E[:, h * OUT_C:(h + 1) * OUT_C],
                    sm_s[0:1, 128:128 + N_NODES],
                    sm_s[0:1, 228 + h * OUT_C:228 + (h + 1) * OUT_C],
                    start=False,
                    stop=True,
                )

            # ---- AllReduce the [100,120] partial embeddings ----
            cc_dt = f32
            cc_in_s = sp.tile([N_NODES, EMBW], cc_dt, tag="cc_in_s")
            nc.vector.tensor_copy(cc_in_s[:, :], psum_E[:, :])
            cc_in_d = dp.tile([N_NODES, EMBW], cc_dt, tag="cc_in_d")
            cc_out_d = dp.tile([N_NODES, EMBW], cc_dt, tag="cc_out_d",
                               addr_space="Shared")
            nc.sync.dma_start(cc_in_d[:, :], cc_in_s[:, :])
            nc.gpsimd.collective_compute(
                "AllReduce",
                mybir.AluOpType.add,
                replica_groups=[list(range(NCORES))],
                ins=[cc_in_d.opt()],
                outs=[cc_out_d.opt()],
            )
            # split the readback: the actor half gates the FC matmul chain, so
            # land it first (in two pieces so tanh/FC can start on the first
            # columns); the critic half proceeds in parallel on DVE while the
            # FC runs on TensorE
            HA = OUT_C // 2
            emb_sum = sp.tile([N_NODES, EMBW], cc_dt, tag="emb_sum")
            nc.sync.dma_start(emb_sum[:, 0:HA], cc_out_d[:, 0:HA])
            nc.sync.dma_start(emb_sum[:, HA:OUT_C], cc_out_d[:, HA:OUT_C])
            nc.sync.dma_start(emb_sum[:, OUT_C:EMBW], cc_out_d[:, OUT_C:EMBW])

            # ---- vnr row: [1,120] = v @ vnr_w + sum(vnr_b), broadcast ----
            psum_v1 = pp.tile([1, EMBW], f32, tag="v1")
            nc.tensor.matmul(psum_v1[:, :], sm_s[0:6, 120:121],
                             sm_s[0:6, 0:EMBW], start=True, stop=True)
            v_row = sp.tile([1, EMBW], f32, tag="v_row")
            nc.vector.tensor_copy(v_row[:, :], psum_v1[:, :])
            psum_vb = pp.tile([N_NODES, EMBW], f32, tag="vb")
            nc.tensor.matmul(psum_vb[:, :], sm_s[0:1, 128:128 + N_NODES],
                             v_row[:, :], start=True, stop=True)

            # ---- tanh + vnr add; actor half in bf16, critic half in f32 ----
            emb_t = sp.tile([N_NODES, EMBW], f32, tag="emb_t")
            emb_a16 = sp.tile([N_NODES, OUT_C], FC_DT, tag="emb_a16")
            for lo, hi in ((0, HA), (HA, OUT_C)):
                nc.scalar.activation(emb_t[:, lo:hi], emb_sum[:, lo:hi],
                                     mybir.ActivationFunctionType.Tanh)
                nc.vector.tensor_add(emb_a16[:, lo:hi], emb_t[:, lo:hi],
                                     psum_vb[:, lo:hi])
            nc.scalar.activation(emb_t[:, OUT_C:EMBW], emb_sum[:, OUT_C:EMBW],
                                 mybir.ActivationFunctionType.Tanh)
            emb_c32 = sp.tile([N_NODES, OUT_C], f32, tag="emb_c32")
            nc.vector.tensor_add(emb_c32[:, :], emb_t[:, OUT_C:EMBW],
                                 psum_vb[:, OUT_C:EMBW])

            # ---- critic value in f32: per-node dot on DVE, then a [1,1]
            # matmul against a ones column to reduce across partitions ----
            tt_s = sp.tile([N_NODES, OUT_C], f32, tag="tt_s")
            dot_col = sp.tile([N_NODES, 1], f32, tag="dot_col")
            nc.vector.tensor_mul(tt_s[:, :], emb_c32[:, :], fcc_s[:, :])
            nc.vector.reduce_sum(dot_col[:, :], tt_s[:, :],
                                 axis=mybir.AxisListType.X)
            psum_val = pp.tile([1, 1], f32, tag="valps")

            # ---- actor logits: 60 bf16 matmuls contracting over nodes; the
            # [1,1] value matmul is interleaved mid-loop (its DVE inputs are
            # ready by then) so it doesn't queue behind the whole chain ----
            psum_fc = pp.tile([1, ACTION], f32, tag="fcps")
            for c in range(OUT_C):
                nc.tensor.matmul(
                    psum_fc[:, :],
                    emb_a16[:, c:c + 1],
                    fc_s[:, c * ACTION:(c + 1) * ACTION],
                    start=(c == 0),
                    stop=(c == OUT_C - 1),
                )
                if c == 20:
                    nc.tensor.matmul(psum_val[:, :], dot_col[:, :],
                                     ones_col[:, :], start=True, stop=True)
            out_s = sp.tile([1, FCW], f32, tag="out_s")
            nc.vector.tensor_add(out_s[:, 0:ACTION], sm_s[0:1, 348:348 + ACTION],
                                 psum_fc[:, :])
            nc.vector.tensor_add(out_s[:, ACTION:FCW], sm_s[0:1, 448:449],
                                 psum_val[:, :])
            nc.sync.dma_start(out[:, :], out_s[:, :])

    nc.finalize()
    return nc


def _prep_inputs(substrate_features, substrate_edge_index, vnr_features,
                 actor_cheb_w, actor_cheb_b, critic_cheb_w, critic_cheb_b,
                 actor_vnr_w, actor_vnr_b, critic_vnr_w, critic_vnr_b,
                 actor_fc_w, actor_fc_b, critic_fc_w, critic_fc_b):
    f = np.float32
    x = np.asarray(substrate_features, f)
    ei = np.asarray(substrate_edge_index)
    v = np.asarray(vnr_features, f).reshape(-1)
    aw = np.asarray(actor_cheb_w, f)
    cw = np.asarray(critic_cheb_w, f)

    # L_hat from edge_index, exactly mirroring the reference formula
    src = np.asarray(ei[0], np.int64)
    dst = np.asarray(ei[1], np.int64)
    deg = np.bincount(src, minlength=N_NODES).astype(f)
    dinv = np.where(deg > 0, 1.0 / np.sqrt(np.maximum(deg, 1.0)), 0.0).astype(f)
    wn = (-dinv[src] * dinv[dst]).astype(np.float64)
    L = np.zeros((N_NODES, N_NODES), np.float64)
    np.add.at(L, (dst, src), wn)
    A = np.stack([np.eye(N_NODES), L, 2.0 * (L @ L) - np.eye(N_NODES)])
    aT_mat = np.ascontiguousarray(
        np.concatenate([A[k].T for k in range(3)], axis=1).astype(f))

    smalls = np.zeros((SM_R, SM_C), f)
    smalls[0:3, :EMBW] = np.concatenate(
        [np.asarray(actor_vnr_w, f), np.asarray(critic_vnr_w, f)], axis=1)
    smalls[3:6, :EMBW] = np.concatenate(
        [np.asarray(actor_vnr_b, f), np.asarray(critic_vnr_b, f)], axis=1)
    smalls[0:3, 120] = v[:3]
    smalls[3:6, 120] = 1.0
    smalls[0, 128:228] = 1.0
    smalls[0, 228:348] = np.concatenate(
        [np.asarray(actor_cheb_b, f), np.asarray(critic_cheb_b, f)]) / NCORES
    smalls[0, 348:348 + ACTION] = np.asarray(actor_fc_b, f)
    smalls[0, 448] = np.asarray(critic_fc_b, f)[0]

    # actor fc reshaped to [n, c*100 + a]; critic fc to [n, c]
    fc_dev = np.ascontiguousarray(
        np.asarray(actor_fc_w, f).reshape(N_NODES, OUT_C * ACTION)
    ).astype(mybir.dt.np(FC_DT))
    fcc_dev = np.ascontiguousarray(
        np.asarray(critic_fc_w, f).reshape(N_NODES, OUT_C))

    main_np = mybir.dt.np(MAIN_DT)
    in_maps = []
    for i in range(NCORES):
        sl = slice(i * CPR, (i + 1) * CPR)
        # pack to SBUF layout: partition p holds k-tile t at cols t*M:(t+1)*M
        xT_i = x[:, sl].T.astype(main_np)            # [8192, 100]
        xp_i = np.ascontiguousarray(
            xT_i.reshape(NKT, 128, N_NODES).transpose(1, 0, 2)
            .reshape(128, NKT * N_NODES))
        w_i = np.concatenate(
            [aw[k, sl, :] for k in range(3)] + [cw[k, sl, :] for k in range(3)],
            axis=1).astype(main_np)                  # [8192, 360]
        wp_i = np.ascontiguousarray(
            w_i.reshape(NKT, 128, W_ALL).transpose(1, 0, 2)
            .reshape(128, NKT * W_ALL))
        in_maps.append({
            "xT": xp_i,
            "w": wp_i,
            "aT": aT_mat,
            "fc": fc_dev,
            "fcc": fcc_dev,
            "smalls": smalls,
        })
    return in_maps


def _run(in_maps, **kwargs):
    if "nc" not in _CACHE:
        _CACHE["nc"] = _build()
    return run_bass_kernel_spmd(_CACHE["nc"], in_maps, core_ids=list(range(NCORES)),
                                **kwargs)


def kernel(**inputs):
    in_maps = _prep_inputs(**inputs)
    res = _run(in_maps)
    o = res.results[0]["out"]
    logits = np.ascontiguousarray(o[0:1, :ACTION], dtype=np.float32)
    values = np.ascontiguousarray(o[0:1, ACTION:ACTION + 1], dtype=np.float32)
    return logits, values
